# revision 33
# baseline (speedup 1.0000x reference)
"""GCN 2-layer encoder on 8 Trainium2 NeuronCores — sharded design v4.

Key identity: segment_sum(norm * (x@W)[src]) == segment_sum(norm * x[src]) @ W,
so each layer gathers RAW features per edge (per-edge norm folded into the
gathered stage tiles via a broadcast multiply), aggregates into
[own_nodes, 128] via one-hot matmuls, and applies the dense 128x128 transform
AFTER aggregation. Layer 1 gathers x directly as f32 512B rows (full DMA
descriptor rate, no table build); layer 2 gathers the allgathered bf16 z.

Each core aggregates only its OWN 12500 dst nodes (~215k edges) for both
layers. Between the layers the z activations are shared with FOUR chunked
AllGathers (6.5MB each) so the collectives overlap with the tail of layer-1
compute and the head of layer-2 gathering.

Schedule: per-core edges bucketed by (dst window, src chunk) with per-window
x128 padding unified across cores, so every 128-slot chunk belongs to exactly
one dst window (one matmul per chunk, no per-op tables). Within a segment
chunks are round-robin interleaved across the block's 12 windows so
consecutive matmuls hit different PSUM banks. One-hot S matrices are built in
batches of G chunks with a single broadcast is_equal tensor_tensor.

Node relabeling: own slice padded to 12544 rows (98 windows), split into 4
AG chunks of [25,25,24,24] windows; table position of node (c,l) =
base_k + c*rows_k + (l - rstart_k). Gather indices are int16 offsets within
one AG chunk's table (max 25600 < 32767).

Collectives cannot sit inside a hardware For_i loop, so the repeat loop is
Python-unrolled.
"""
import os
import sys

sys.path.insert(0, "/opt/trn_rl_repo")
import numpy as np
import ml_dtypes

import concourse.tile as tile
from concourse import bacc, mybir, library_config
from concourse.bass_utils import run_bass_kernel_spmd

N_NODES = 100000
N_CORES = 8
S = N_NODES // N_CORES          # 12500 own nodes per core
SP = 12544                      # padded own slice (98 windows x 128)
D = 128
NW = SP // 128                  # 98 own dst windows
WPT = 12                        # windows per block (3 PSUM banks x 4)
NWB = (NW + WPT - 1) // WPT     # 9 blocks
NK = 4                          # AG chunks
WK = [25, 25, 24, 24]           # windows per AG chunk
WSTART = [0, 25, 50, 74]
ROWS_K = [w * 128 for w in WK]              # per-core rows per chunk
RSTART = [0, 3200, 6400, 9472]
BASE_K = [0, 25600, 51200, 75776]           # global table base per chunk
CALL = int(os.environ.get("KERNEL_CALL", "1024"))
G = 16                          # one-hot build batch (chunks per DVE op)
f32 = mybir.dt.float32
bf16 = mybir.dt.bfloat16
i16 = mybir.dt.int16


def _pad128(n):
    return ((n + 127) // 128) * 128


def _wrap_idx(gidx_flat):
    """[slots] int16 -> [128, slots/16] wrapped+replicated for dma_gather."""
    a = gidx_flat.reshape(-1, 16).T
    return np.tile(a, (8, 1)).copy()


def _chunk_of_local(l):
    """AG chunk index of own-local node l."""
    return np.searchsorted(RSTART, l, side="right") - 1


def _prep(edge_index):
    src = np.asarray(edge_index[0], dtype=np.int64)
    dst = np.asarray(edge_index[1], dtype=np.int64)
    deg = (np.bincount(dst, minlength=N_NODES) + 1).astype(np.float64)
    dinv = 1.0 / np.sqrt(deg)

    loop = np.arange(N_NODES, dtype=np.int64)
    src_all = np.concatenate([src, loop])
    dst_all = np.concatenate([dst, loop])
    norm_all = dinv[src_all] * dinv[dst_all]
    sc = src_all // S                       # src owner core
    sl = src_all % S                        # src local id
    sk = _chunk_of_local(sl)                # src AG chunk
    spos = np.take(BASE_K, sk) + sc * np.take(ROWS_K, sk) \
        + (sl - np.take(RSTART, sk))        # global table position
    sidx = spos - np.take(BASE_K, sk)       # int16 offset within chunk table

    core = dst_all // S
    # per (core, window, k): sorted edge lists
    per = []
    cnt = np.zeros((N_CORES, NW, NK), np.int64)
    for c in range(N_CORES):
        m = core == c
        dl = dst_all[m] - c * S
        w = dl // 128
        k = sk[m]
        order = np.lexsort((sidx[m], k, w))
        per.append({
            "w": w[order], "k": k[order], "dl": dl[order],
            "gi": sidx[m][order], "no": norm_all[m][order],
        })
        np.add.at(cnt[c], (w, k), 1)
    La = np.zeros((NW, NK), np.int64)
    for w in range(NW):
        for k in range(NK):
            La[w, k] = _pad128(int(cnt[:, w, k].max()))
    # chunk emission order per segment (bb, k): round-robin across windows
    seg_list = [(bb, k) for bb in range(NWB) for k in range(NK)]
    seg_base, seg_len = {}, {}
    chunk_meta = []                      # (bb, k, w, r) in emission order
    off = 0
    for (bb, k) in seg_list:
        seg_base[(bb, k)] = off
        wins = list(range(bb * WPT, min(NW, (bb + 1) * WPT)))
        nmax = max(La[w, k] // 128 for w in wins)
        cnt_emitted = 0
        for r in range(nmax):
            for w in wins:
                if r < La[w, k] // 128:
                    chunk_meta.append((bb, k, w, r))
                    cnt_emitted += 1
        ln = cnt_emitted * 128
        seg_len[(bb, k)] = ln
        off += ln
    tot = off
    nch = tot // 128
    # slot-fill per core
    gidx = np.zeros((N_CORES, tot), np.int16)
    dstm = np.full((N_CORES, nch, 128), -1000.0, np.float32)
    ddv = np.zeros((N_CORES, nch, 128), np.float32)
    # per-core edge run boundaries for each (w,k)
    for c in range(N_CORES):
        p = per[c]
        key = p["w"] * NK + p["k"]
        bounds = np.flatnonzero(np.diff(key)) + 1
        starts = np.concatenate([[0], bounds])
        ends = np.concatenate([bounds, [len(key)]])
        run = {int(key[s0]): (s0, e0) for s0, e0 in zip(starts, ends)}
        for ch, (bb, k, w, r) in enumerate(chunk_meta):
            kk = w * NK + k
            if kk not in run:
                continue
            s0, e0 = run[kk]
            a = s0 + r * 128
            b = min(s0 + (r + 1) * 128, e0)
            if a >= b:
                continue
            n = b - a
            gidx[c, ch * 128:ch * 128 + n] = p["gi"][a:b]
            dstm[c, ch, :n] = p["dl"][a:b] - 128.0 * w
            ddv[c, ch, :n] = p["no"][a:b]
    dSem = np.ascontiguousarray(
        dstm.transpose(0, 2, 1)).astype(ml_dtypes.bfloat16)   # [C,128,nch]
    ddT = np.ascontiguousarray(ddv.transpose(0, 2, 1))        # [C,128,nch]
    # ops = chunks; start/stop flags per (block, bank)
    ops_by_seg = {skey: [] for skey in seg_list}
    flags = {}
    for ch, (bb, k, w, r) in enumerate(chunk_meta):
        ops_by_seg[(bb, k)].append([ch, w, False, False])
    first_bk, last_bk = {}, {}
    i = 0
    for skey in seg_list:
        for op in ops_by_seg[skey]:
            ch, w = op[0], op[1]
            bb = skey[0]
            bk = (bb, (w - bb * WPT) // 4)
            if bk not in first_bk:
                first_bk[bk] = op
            last_bk[bk] = op
            i += 1
    for op in first_bk.values():
        op[2] = True
    for op in last_bk.values():
        op[3] = True
    return {
        "seg_list": seg_list, "seg_base": seg_base, "seg_len": seg_len,
        "tot": tot, "nch": nch,
        "gidx": gidx, "dSem": dSem, "dd": ddT, "ops_by_seg": ops_by_seg,
    }


def _build(sch, repeat=1, phases="BD", ag=True):
    nc = bacc.Bacc("TRN2", target_bir_lowering=False, debug=False,
                   num_devices=N_CORES, num_swdge_queues=4)
    xq = [nc.dram_tensor(f"xq{k}", [8 * ROWS_K[k], D], f32,
                         kind="ExternalInput") for k in range(NK)]
    W1b = nc.dram_tensor("W1b", [128, 128], bf16, kind="ExternalInput")
    W2b = nc.dram_tensor("W2b", [128, 128], bf16, kind="ExternalInput")
    b1b = nc.dram_tensor("b1b", [128, 128], f32, kind="ExternalInput")
    b2b = nc.dram_tensor("b2b", [128, 128], f32, kind="ExternalInput")
    iotab = nc.dram_tensor("iotab", [128, 128], bf16, kind="ExternalInput")
    gx = nc.dram_tensor("gx", [128, sch["tot"] // 16], i16,
                        kind="ExternalInput")
    dSe = nc.dram_tensor("dSe", [128, sch["nch"]], bf16,
                         kind="ExternalInput")
    dd = nc.dram_tensor("dd", [128, sch["nch"]], bf16, kind="ExternalInput")
    out = nc.dram_tensor("out", [S, D], f32, kind="ExternalOutput")

    agin = [nc.dram_tensor(f"agin{k}", [ROWS_K[k], D], bf16)
            for k in range(NK)]
    agout = [nc.dram_tensor(f"agout{k}", [8 * ROWS_K[k], D], bf16,
                            addr_space="Shared") for k in range(NK)]

    wpwb = [min(NW - bb * WPT, WPT) for bb in range(NWB)]

    with tile.TileContext(nc) as tc:
        with (
            tc.tile_pool(name="cst", bufs=1) as cst,
            tc.tile_pool(name="ps", bufs=2, space="PSUM") as php,
            tc.tile_pool(name="st", bufs=4) as stp,
            tc.tile_pool(name="stc", bufs=18) as stcp,
            tc.tile_pool(name="gxs", bufs=2) as gxp,
            tc.tile_pool(name="oh", bufs=3) as ohp,
            tc.tile_pool(name="bank", bufs=1, space="PSUM") as bankp,
            tc.tile_pool(name="fl", bufs=2) as flp,
            tc.tile_pool(name="zb", bufs=1) as zbp,
        ):
            nc.gpsimd.load_library(library_config.mlp)

            W1_sb = cst.tile([128, 128], bf16, tag="W1")
            W2_sb = cst.tile([128, 128], bf16, tag="W2")
            b1_sb = cst.tile([128, 128], f32, tag="b1")
            b2_sb = cst.tile([128, 128], f32, tag="b2")
            iota_sb = cst.tile([128, 128], bf16, tag="iota")
            dS_sb = cst.tile([128, sch["nch"]], bf16, tag="dS")
            dd_sb = cst.tile([128, sch["nch"]], bf16, tag="dd")
            nc.sync.dma_start(W1_sb[:], W1b[:])
            nc.sync.dma_start(W2_sb[:], W2b[:])
            nc.sync.dma_start(b1_sb[:], b1b[:])
            nc.sync.dma_start(b2_sb[:], b2b[:])
            nc.sync.dma_start(iota_sb[:], iotab[:])
            nc.sync.dma_start(dS_sb[:], dSe[:])
            nc.sync.dma_start(dd_sb[:], dd[:])
            banks = [bankp.tile([128, 512], f32, tag=f"bk{i}",
                                name=f"bank{i}") for i in range(6)]

            qctr = [0]
            blkctr = [0]

            def aggregate(layer1):
                acc = zbp.tile([128, SP], bf16, tag="acc")
                for bb in range(NWB):
                    par = blkctr[0] % 2
                    blkctr[0] += 1
                    blk_base = sch["seg_base"][(bb, 0)]
                    blk_len = sum(sch["seg_len"][(bb, kq)]
                                  for kq in range(NK))
                    assert blk_len // 16 <= 2368, blk_len
                    gx_t = gxp.tile([128, 2368], i16, tag="gx")
                    nc.sync.dma_start(
                        gx_t[:, :blk_len // 16],
                        gx.ap()[:, blk_base // 16:
                                (blk_base + blk_len) // 16])
                    for kq in range(NK):
                        Lseg = sch["seg_len"][(bb, kq)]
                        if Lseg == 0:
                            continue
                        base = sch["seg_base"][(bb, kq)]
                        ncalls = (Lseg + CALL - 1) // CALL
                        assert ncalls <= 13, (Lseg, CALL)
                        stages = []
                        for k in range(ncalls):
                            cl = min(CALL, Lseg - CALL * k)
                            nc_ = cl // 128
                            ch0 = (base + CALL * k) // 128
                            dd_b = dd_sb[:, ch0:ch0 + nc_] \
                                .unsqueeze(2).broadcast_to([128, nc_, 128])
                            if layer1:
                                stg = stp.tile([128, CALL // 128, 128], f32,
                                               tag="stg")
                                src_ap = xq[kq].ap()
                            else:
                                stg = stp.tile([128, CALL // 128, 128], bf16,
                                               tag="str")
                                src_ap = agout[kq].ap()
                            off = base - blk_base + CALL * k
                            nc.gpsimd.dma_gather(
                                stg[:, :nc_, :], src_ap,
                                gx_t[:, off // 16:(off + cl) // 16],
                                cl, cl, 128, queue_num=qctr[0] % 4)
                            qctr[0] += 1
                            stb = stcp.tile([128, CALL // 128, 128],
                                            bf16, tag="stb")
                            nc.vector.tensor_tensor(
                                stb[:, :nc_, :], stg[:, :nc_, :], dd_b,
                                op=mybir.AluOpType.mult)
                            stages.append(stb)
                        ops = sch["ops_by_seg"][(bb, kq)]
                        sbatch = None
                        for ei, op in enumerate(ops):
                            ch, wv, st_f, sp_f = op
                            li = ei % G
                            if li == 0:
                                g = min(G, len(ops) - ei)
                                ch0 = base // 128 + ei
                                sbatch = ohp.tile([128, G, 128], bf16,
                                                  tag="S")
                                io_b = iota_sb[:].unsqueeze(1) \
                                    .broadcast_to([128, g, 128])
                                dS_b = dS_sb[:, ch0:ch0 + g].unsqueeze(2) \
                                    .broadcast_to([128, g, 128])
                                nc.vector.tensor_tensor(
                                    sbatch[:, :g, :], io_b, dS_b,
                                    op=mybir.AluOpType.is_equal)
                            jseg = ch - base // 128
                            k = jseg // (CALL // 128)
                            jc = jseg % (CALL // 128)
                            wl = wv - bb * WPT
                            bank = banks[par * 3 + wl // 4]
                            bsl = bank[:, 128 * (wl % 4):128 * (wl % 4 + 1)]
                            nc.tensor.matmul(
                                bsl, lhsT=stages[k][:, jc, :],
                                rhs=sbatch[:, li, :],
                                start=st_f, stop=sp_f)
                    # flush block: per-bank psum->acc copies, dense transform
                    nwin = wpwb[bb]
                    for bk in range((nwin + 3) // 4):
                        nb = min(4, nwin - 4 * bk)
                        bank = banks[par * 3 + bk]
                        wv0 = bb * WPT + 4 * bk
                        nc.scalar.activation(
                            acc[:, 128 * wv0:128 * (wv0 + nb)],
                            bank[:, :128 * nb],
                            mybir.ActivationFunctionType.Copy)
                    w_sb = W1_sb if layer1 else W2_sb
                    b_sb = b1_sb if layer1 else b2_sb
                    ob = flp.tile([128, WPT * 128], bf16 if layer1 else f32,
                                  tag="ob1" if layer1 else "ob2")
                    b4_b = b_sb[:].unsqueeze(1).broadcast_to([128, 4, 128])
                    for bk in range((nwin + 3) // 4):
                        nb = min(4, nwin - 4 * bk)
                        ps = php.tile([128, 4, 128], f32, tag="php")
                        for j in range(nb):
                            wv = bb * WPT + 4 * bk + j
                            nc.tensor.matmul(
                                ps[:, j, :],
                                lhsT=acc[:, 128 * wv:128 * (wv + 1)],
                                rhs=w_sb[:], start=True, stop=True)
                        osl = ob[:, 512 * bk:512 * bk + 128 * nb].rearrange(
                            "p (t f) -> p t f", f=128)
                        if layer1:
                            t1 = flp.tile([128, 4, 128], f32, tag="t1")
                            nc.vector.tensor_tensor(
                                t1[:, :nb, :], ps[:, :nb, :], b4_b[:, :nb, :],
                                op=mybir.AluOpType.add)
                            nc.vector.tensor_scalar(
                                osl, t1[:, :nb, :], 0.0, None,
                                op0=mybir.AluOpType.max)
                        else:
                            nc.vector.tensor_tensor(
                                osl, ps[:, :nb, :], b4_b[:, :nb, :],
                                op=mybir.AluOpType.add)
                    w0 = bb * WPT
                    if layer1:
                        # write into agin chunks, splitting at boundaries
                        i = 0
                        while i < nwin:
                            wv = w0 + i
                            kq = next(kk for kk in range(NK)
                                      if WSTART[kk] <= wv
                                      < WSTART[kk] + WK[kk])
                            nblk = min(nwin - i,
                                       WSTART[kq] + WK[kq] - wv)
                            r0 = 128 * (wv - WSTART[kq])
                            nc.sync.dma_start(
                                agin[kq].ap()[r0:r0 + 128 * nblk, :]
                                .rearrange("(t p) f -> p t f", p=128),
                                ob[:, 128 * i:128 * (i + nblk)].rearrange(
                                    "p (t f) -> p t f", f=128))
                            i += nblk
                    else:
                        rows = min(S, 128 * (w0 + nwin)) - 128 * w0
                        nfull = rows // 128
                        if nfull:
                            nc.sync.dma_start(
                                out.ap()[128 * w0:128 * (w0 + nfull), :]
                                .rearrange("(t p) f -> p t f", p=128),
                                ob[:, :128 * nfull].rearrange(
                                    "p (t f) -> p t f", f=128))
                        rem = rows - 128 * nfull
                        if rem:
                            nc.sync.dma_start(
                                out.ap()[128 * (w0 + nfull):
                                         128 * (w0 + nfull) + rem, :],
                                ob[0:rem, 128 * nfull:128 * (nfull + 1)])

            for r in range(repeat):
                if "B" in phases:
                    with nc.named_scope("phB"):
                        aggregate(layer1=True)
                if ag:
                    with nc.named_scope("phAG"):
                        for k in range(NK):
                            nc.gpsimd.collective_compute(
                                "AllGather", mybir.AluOpType.bypass,
                                replica_groups=[list(range(N_CORES))],
                                ins=[agin[k][:].opt()],
                                outs=[agout[k][:].opt()])
                if "D" in phases:
                    with nc.named_scope("phD"):
                        aggregate(layer1=False)

    # Align each gather's SWDGE queue with its Tile-assigned DMASW sem lane
    # (queue = lane % 4): the scheduler may reorder gathers after my
    # round-robin queue choice, and a lane semaphore must not be updated
    # from two different queues while in flight.
    from concourse.tile_sem_assignment import PROC_NAME_TO_IDX
    lane0 = PROC_NAME_TO_IDX["DMASW0"]
    for blk in nc.m.functions[0].blocks:
        for inst in blk.instructions:
            if type(inst).__name__ == "InstDMAGatherAnt":
                inst.queue_num = (inst.bass_scheduled_proc - lane0) % 4
    nc.compile()
    return nc


def _make_in_maps(x, W1, b1, W2, b2, sch):
    xv = np.asarray(x, np.float32)
    xqs = {}
    for k in range(NK):
        t = np.zeros((8 * ROWS_K[k], D), np.float32)
        for c in range(N_CORES):
            l0, l1 = RSTART[k], min(RSTART[k] + ROWS_K[k], S)
            t[c * ROWS_K[k]:c * ROWS_K[k] + (l1 - l0)] = \
                xv[c * S + l0:c * S + l1]
        xqs[f"xq{k}"] = t
    iota = np.broadcast_to(np.arange(128, dtype=np.float32),
                           (128, 128)).astype(ml_dtypes.bfloat16)
    common = {
        **xqs,
        "W1b": np.asarray(W1, np.float32).astype(ml_dtypes.bfloat16),
        "W2b": np.asarray(W2, np.float32).astype(ml_dtypes.bfloat16),
        "b1b": np.broadcast_to(np.asarray(b1, np.float32), (128, 128)).copy(),
        "b2b": np.broadcast_to(np.asarray(b2, np.float32), (128, 128)).copy(),
        "iotab": np.ascontiguousarray(iota),
    }
    in_maps = []
    for c in range(N_CORES):
        in_maps.append({
            **common,
            "gx": _wrap_idx(sch["gidx"][c]),
            "dSe": sch["dSem"][c],
            "dd": sch["dd"][c].astype(ml_dtypes.bfloat16),
        })
    return in_maps


def kernel(x, edge_index, W1, b1, W2, b2):
    sch = _prep(edge_index)
    nc = _build(sch, repeat=int(os.environ.get("KERNEL_REPEAT", "1")))
    in_maps = _make_in_maps(x, W1, b1, W2, b2, sch)
    res = run_bass_kernel_spmd(nc, in_maps, core_ids=list(range(N_CORES)))
    return np.concatenate([res.results[c]["out"] for c in range(N_CORES)], 0)
